# revision 1
# baseline (speedup 1.0000x reference)
"""Trainium2 Bass kernel for the KNet-style recurrent chain (batch=1).

Strategy (memory-bound problem, ~353MB of fp32 weights):
  - The small GRU chain + small FCs (~35MB) are REPLICATED on all 8 cores.
  - FC2 (the big Kalman-gain MLP: W2a [46080,1152], W2b [576,46080]) is
    tensor-parallel: each core gets 5760 rows of W2a and the matching 5760
    columns of W2b, computes a partial y [576]; the host sums the 8 partials
    and adds b2b (the "all-reduce" done on host).
  - Every matvec y = W @ x runs on the TensorEngine in WEIGHT-MOVING form:
        out[1, N] (+)= x_chunk[K, 1].T @ W.T_chunk[K, N]
    i.e. the tiny activation chunk is the stationary operand (fast fp32
    load) and the pre-transposed weights stream as the moving operand
    (~430ns per [128, 512] fp32 block, ~611 GB/s — above the per-core HBM
    rate).  Keeping weights stationary instead costs ~350ns per [128,128]
    tile (fp32 weight load), 3x too slow.
  - Matvec outputs live in free-layout [1, M] (one partition); elementwise
    GRU math happens there; PE transpose-mode matmuls ([1,128] -> [128,1],
    ~330ns) rebuild the partition-layout [128, ceil(d/128)] tiles consumed
    as the next layer's stationary chunks.
  - PSUM accumulation: start=True clears has_written for the WHOLE target
    bank, so it is set only on the first matmul into each bank; later
    first-writes to an element overwrite because has_written=0.
"""

import sys

sys.path.insert(0, "/opt/trn_rl_repo")

import numpy as np

NCORES = 8
H = 576                      # hidden size of all three GRUs
D2_HID, D2_IN, D2_OUT = 46080, 1152, 576
MSH = D2_HID // NCORES       # 5760 rows of W2a per core
NM2 = MSH // 128             # 45 output chunks per core
STRIPE = 512                 # FC2a output stripe width
W2B_GRP = 3                  # FC2b K-blocks per DMA

F32 = np.float32


def _ncols(d):
    return (d + 127) // 128


def _nsplits(m):
    """split free dim at 512 boundaries (= PSUM bank boundaries)."""
    return [(n0, min(512, m - n0)) for n0 in range(0, m, 512)]


_CACHE = {}


class _Vec:
    """An activation vector in SBUF P-layout [128, ncols]."""

    def __init__(self, tile, d):
        self.tile = tile
        self.d = d

    def chunks(self):
        for c in range(_ncols(self.d)):
            sz = min(128, self.d - c * 128)
            yield self.tile[0:sz, c : c + 1], sz


def _build_program(dbg=False):
    import concourse.bass as bass  # noqa: F401
    from concourse import bacc, mybir
    import concourse.tile as tile

    f32 = mybir.dt.float32
    f32r = mybir.dt.float32r
    AF = mybir.ActivationFunctionType

    nc = bacc.Bacc(
        "TRN2", target_bir_lowering=False, debug=False, num_devices=NCORES
    )

    def din(name, shape, dt=f32):
        return nc.dram_tensor(name, list(shape), dt, kind="ExternalInput")

    # --- dram inputs: activation vectors ---
    d_x5 = din("x5", (24, 1), f32r)
    d_x6 = din("x6", (24, 1), f32r)
    d_obs = din("obs", (48, 1), f32r)
    d_hq = din("h_q", (128, 5), f32r)      # P-layout (matvec operand)
    d_hsig = din("h_sig", (128, 5), f32r)
    d_hs = din("h_s", (128, 5), f32r)
    d_hq_f = din("h_q_f", (1, H))    # free-layout (elementwise operand)
    d_hsig_f = din("h_sig_f", (1, H))
    d_hs_f = din("h_s_f", (1, H))

    # --- dram inputs: weights, host-stored as W.T [K, M] row-major ---
    wshapes = {
        "w5": (24, 480), "w6": (24, 480), "w7": (48, 960), "w1": (576, 576),
        "wrz_q": (1056, 1152), "win_q": (480, 576), "whn_q": (576, 576),
        "wrz_sig": (1632, 1152), "win_sig": (1056, 576), "whn_sig": (576, 576),
        "wrz_s": (2112, 1152), "win_s": (1536, 576), "whn_s": (576, 576),
        "w2a": (D2_IN, MSH), "w2b": (MSH, D2_OUT),
    }
    dw = {k: din(k, v, f32r) for k, v in wshapes.items()}

    # --- dram inputs: biases in free-layout [1, M] ---
    bshapes = {
        "b5": 480, "b6": 480, "b7": 960, "b1": H,
        "brz_q": 1152, "bin_q": H, "bhn_q": H,
        "brz_sig": 1152, "bin_sig": H, "bhn_sig": H,
        "brz_s": 1152, "bin_s": H, "bhn_s": H,
    }
    db = {k: din(k, (1, v)) for k, v in bshapes.items()}

    d_b2a = din("b2a", (1, MSH))
    d_y = nc.dram_tensor("y", [1, D2_OUT], f32, kind="ExternalOutput")

    dbg_outs = {}

    def _dbg(name, tile_ap, shape):
        if not dbg:
            return
        dt = nc.dram_tensor(f"dbg_{name}", list(shape), f32,
                            kind="ExternalOutput")
        nc.sync.dma_start(out=dt[:], in_=tile_ap.bitcast(f32))
        dbg_outs[name] = dt

    with tile.TileContext(nc) as tc:
        with (
            tc.tile_pool(name="const", bufs=1) as constp,
            tc.tile_pool(name="vecs", bufs=1) as vecp,
            tc.tile_pool(name="smallw", bufs=3) as swp,
            tc.tile_pool(name="bigw", bufs=3) as bigp,
            tc.tile_pool(name="w2bp", bufs=2) as w2bp,
            tc.tile_pool(name="ps", bufs=1, space="PSUM") as psp,
        ):
            def load_const(dram, shape, name, dt=f32):
                t = constp.tile(list(shape), dt, name=name, tag=name)
                nc.sync.dma_start(out=t, in_=dram[:])
                return t

            x5 = _Vec(load_const(d_x5, (24, 1), "t_x5", f32r), 24)
            x6 = _Vec(load_const(d_x6, (24, 1), "t_x6", f32r), 24)
            obs = _Vec(load_const(d_obs, (48, 1), "t_obs", f32r), 48)
            h_q = _Vec(load_const(d_hq, (128, 5), "t_hq", f32r), H)
            h_sig = _Vec(load_const(d_hsig, (128, 5), "t_hsig", f32r), H)
            h_s = _Vec(load_const(d_hs, (128, 5), "t_hs", f32r), H)
            hf = {
                "q": load_const(d_hq_f, (1, H), "t_hq_f"),
                "sig": load_const(d_hsig_f, (1, H), "t_hsig_f"),
                "s": load_const(d_hs_f, (1, H), "t_hs_f"),
            }
            bt = {
                k: load_const(db[k], (1, v), "t_" + k)
                for k, v in bshapes.items()
            }
            ident = constp.tile([1, 1], f32, name="ident", tag="ident")
            nc.vector.memset(ident, 1.0)

            def load_w_chunks(wname, segs, m_out):
                """DMA pre-transposed weights; yield (wt_ap, rhs, ksz)."""
                w = dw[wname]
                chunks = []
                ro = 0
                # cap tile size at ~14KB/partition
                grp = max(1, 14336 // (m_out * 4))
                for v in segs:
                    nb, tail = v.d // 128, v.d % 128
                    rhs_cols = list(v.chunks())
                    for g0 in range(0, nb, grp):
                        gn = min(grp, nb - g0)
                        wt = swp.tile([128, gn, m_out], f32r, tag="sw",
                                      name=f"w_{wname}_{ro}f{g0}", bufs=3)
                        nc.sync.dma_start(
                            out=wt,
                            in_=w[ro + g0 * 128 : ro + (g0 + gn) * 128,
                                  :].rearrange("(b p) m -> p b m", p=128),
                        )
                        for b in range(gn):
                            chunks.append(
                                (wt[:, b, :], rhs_cols[g0 + b][0], 128)
                            )
                    if tail:
                        wtt = swp.tile([tail, m_out], f32r, tag="sw",
                                       name=f"w_{wname}_{ro}t", bufs=3)
                        nc.sync.dma_start(
                            out=wtt, in_=w[ro + nb * 128 : ro + v.d, :]
                        )
                        chunks.append((wtt, rhs_cols[nb][0], tail))
                    ro += v.d
                return chunks

            def matvec_f(wname, segs, m_out, bias_tile, act, out_name,
                         psum_tag, psum_bufs, out_tag=None, out_bufs=2):
                """free-layout matvec: returns sbuf AP [1, m_out] of
                act(W @ concat(segs) + b)."""
                psum = psp.tile([1, max(m_out, 1152)], f32,
                                name=f"ps_{out_name}", tag=psum_tag,
                                bufs=psum_bufs)
                chunks = load_w_chunks(wname, segs, m_out)
                nch = len(chunks)
                for ci, (wt_ap, rhs, ksz) in enumerate(chunks):
                    for n0, nsz in _nsplits(m_out):
                        nc.tensor.matmul(
                            psum[0:1, n0 : n0 + nsz],
                            rhs,
                            wt_ap[0:ksz, n0 : n0 + nsz],
                            start=(ci == 0),
                            stop=(ci == nch - 1),
                            skip_group_check=True,
                        )
                out = vecp.tile([1, m_out], f32, name=out_name,
                                tag=out_tag or out_name,
                                bufs=out_bufs if out_tag else 1)
                nc.vector.tensor_add(out, psum[0:1, 0:m_out], bias_tile)
                if act is not None:
                    nc.scalar.activation(out, out, act)
                return out

            def to_play(free_ap, d, name):
                """transpose free-layout [1, d] -> P-layout [128, ncols]."""
                n_m = _ncols(d)
                ps_t = psp.tile([128, NM2], f32, name=f"pst_{name}",
                                tag="tp", bufs=1)
                for c in range(n_m):
                    csz = min(128, d - c * 128)
                    nc.tensor.matmul(
                        ps_t[0:csz, c : c + 1],
                        free_ap[0:1, c * 128 : c * 128 + csz],
                        ident,
                        is_transpose=True,
                        start=(c == 0),
                        stop=(c == n_m - 1),
                        skip_group_check=True,
                    )
                pl = vecp.tile([128, n_m], f32r, name=name, tag=name)
                nc.vector.tensor_copy(pl, ps_t[:, 0:n_m])
                return _Vec(pl, d)

            def gru(g, x_segs, h, out_name):
                rz = matvec_f(f"wrz_{g}", x_segs + [h], 2 * H,
                              bt[f"brz_{g}"], AF.Sigmoid, f"rz_{g}",
                              "mv1", 1, out_tag="rz_sb")
                gin = matvec_f(f"win_{g}", x_segs, H, bt[f"bin_{g}"], None,
                               f"gin_{g}", "mv1", 1, out_tag="gin_sb")
                ghn = matvec_f(f"whn_{g}", [h], H, bt[f"bhn_{g}"], None,
                               f"ghn_{g}", "mv1", 1, out_tag="ghn_sb")
                # n = tanh(gin + r * ghn);  h' = n + z * (h - n)
                t3 = vecp.tile([1, H], f32, name=f"t3_{g}", tag="t3",
                                bufs=1)
                nc.vector.tensor_mul(t3, rz[0:1, 0:H], ghn)
                nc.vector.tensor_add(t3, gin, t3)
                n_t = vecp.tile([1, H], f32, name=f"n_{g}", tag="n_t",
                                bufs=1)
                nc.scalar.activation(n_t, t3, AF.Tanh)
                t5 = vecp.tile([1, H], f32, name=f"t5_{g}", tag="t5",
                                bufs=1)
                nc.vector.tensor_sub(t5, hf[g], n_t)
                nc.vector.tensor_mul(t5, rz[0:1, H : 2 * H], t5)
                hn = vecp.tile([1, H], f32, name=out_name, tag="hn",
                                bufs=1)
                nc.vector.tensor_add(hn, n_t, t5)
                return hn

            # ---- the chain ----
            out5_f = matvec_f("w5", [x5], 480, bt["b5"], AF.Relu,
                              "out5_f", "mv1", 1, out_tag="vf")
            out5 = to_play(out5_f, 480, "out5")
            _dbg("out5", out5.tile, (128, 4))
            hQ_f = gru("q", [out5], h_q, "hQ_f")
            hQ = to_play(hQ_f, H, "hQ")
            _dbg("hQ", hQ.tile, (128, 5))
            out6_f = matvec_f("w6", [x6], 480, bt["b6"], AF.Relu,
                              "out6_f", "mv1", 1, out_tag="vf")
            out6 = to_play(out6_f, 480, "out6")
            _dbg("out6", out6.tile, (128, 4))
            hSig_f = gru("sig", [hQ, out6], h_sig, "hSig_f")
            hSig = to_play(hSig_f, H, "hSig")
            _dbg("hSig", hSig.tile, (128, 5))
            out1_f = matvec_f("w1", [hSig], H, bt["b1"], AF.Relu,
                              "out1_f", "mv1", 1, out_tag="vf")
            out1 = to_play(out1_f, H, "out1")
            _dbg("out1", out1.tile, (128, 5))
            out7_f = matvec_f("w7", [obs], 960, bt["b7"], AF.Relu,
                              "out7_f", "mv1", 1, out_tag="vf")
            out7 = to_play(out7_f, 960, "out7")
            _dbg("out7", out7.tile, (128, 8))
            hS_f = gru("s", [out1, out7], h_s, "hS_f")
            if dbg:
                hS = to_play(hS_f, H, "hS")
                _dbg("hS", hS.tile, (128, 5))

            # ---- FC2a: h_fc = relu(W2a_shard @ [hSig, hS] + b2a_shard) ----
            # Build in2 = concat(hSig, hS) contiguously in free layout, then
            # transpose to a clean [128, 9] P-layout (1152 = 9*128 exactly).
            # Per output stripe of 512 the whole [1152, 512] weight block
            # arrives as ONE 2.36MB DMA.
            in2_f = vecp.tile([1, D2_IN], f32, name="in2_f", tag="in2_f")
            nc.vector.tensor_copy(in2_f[0:1, 0:H], hSig_f)
            nc.vector.tensor_copy(in2_f[0:1, H : 2 * H], hS_f)
            in2 = to_play(in2_f, D2_IN, "in2t")
            NK2 = D2_IN // 128  # 9
            ps_hfc = psp.tile([128, NM2], f32, name="ps_hfc", tag="tp",
                              bufs=1)
            n_tp = 0
            for m0, nsz in _nsplits(MSH):
                psf = psp.tile([1, STRIPE], f32, name=f"ps_f{m0}",
                               tag="fca", bufs=2)
                b2s = vecp.tile([1, STRIPE], f32, name=f"b2s_{m0}",
                                tag="b2as", bufs=2)
                nc.sync.dma_start(out=b2s[0:1, 0:nsz],
                                  in_=d_b2a[0:1, m0 : m0 + nsz])
                hstr = vecp.tile([1, STRIPE], f32, name=f"hstr_{m0}",
                                 tag="hstr", bufs=2)
                wt = bigp.tile([128, NK2, nsz], f32r, tag="w2a",
                               name=f"w2a_{m0}", bufs=3)
                nc.sync.dma_start(
                    out=wt,
                    in_=dw["w2a"][:, m0 : m0 + nsz].rearrange(
                        "(b p) m -> p b m", p=128
                    ),
                )
                rhs_cols = list(in2.chunks())
                for ci in range(NK2):
                    nc.tensor.matmul(
                        psf[0:1, 0:nsz],
                        rhs_cols[ci][0],
                        wt[:, ci, 0:nsz],
                        start=(ci == 0),
                        stop=(ci == NK2 - 1),
                        skip_group_check=True,
                    )
                # bias + relu into the free-layout accumulator
                nc.vector.tensor_add(
                    hstr[0:1, 0:nsz], psf[0:1, 0:nsz], b2s[0:1, 0:nsz]
                )
                nc.scalar.activation(
                    hstr[0:1, 0:nsz], hstr[0:1, 0:nsz], AF.Relu
                )
                # transpose this stripe into P-layout columns
                for c in range(nsz // 128):
                    col = m0 // 128 + c
                    nc.tensor.matmul(
                        ps_hfc[:, col : col + 1],
                        hstr[0:1, c * 128 : (c + 1) * 128],
                        ident,
                        is_transpose=True,
                        start=(n_tp == 0),
                        stop=(n_tp == NM2 - 1),
                        skip_group_check=True,
                    )
                    n_tp += 1
            h_fc = vecp.tile([128, NM2], f32r, name="h_fc", tag="h_fc")
            nc.vector.tensor_copy(h_fc, ps_hfc)
            _dbg("h_fc", h_fc, (128, NM2))

            # ---- FC2b: y_partial = W2b_shard @ h_fc  (out [1, 576]) ----
            ps512 = psp.tile([1, 512], f32, name="ps_y512", tag="y512",
                             bufs=1)
            ps64 = psp.tile([1, 64], f32, name="ps_y64", tag="y64", bufs=1)
            for g in range(NM2 // W2B_GRP):
                wt = w2bp.tile([128, W2B_GRP, D2_OUT], f32r, tag="w2b",
                               name=f"w2b_{g}", bufs=2)
                r0 = g * W2B_GRP * 128
                nc.sync.dma_start(
                    out=wt,
                    in_=dw["w2b"][r0 : r0 + W2B_GRP * 128, :].rearrange(
                        "(b p) m -> p b m", p=128
                    ),
                )
                for j in range(W2B_GRP):
                    kb = g * W2B_GRP + j
                    lhs = h_fc[:, kb : kb + 1]
                    nc.tensor.matmul(
                        ps512[0:1, :], lhs,
                        wt[:, j, 0:512],
                        start=(kb == 0), stop=(kb == NM2 - 1),
                        skip_group_check=True,
                    )
                    nc.tensor.matmul(
                        ps64[0:1, :], lhs,
                        wt[:, j, 512:576],
                        start=(kb == 0), stop=(kb == NM2 - 1),
                        skip_group_check=True,
                    )
            y_sb = constp.tile([1, D2_OUT], f32, name="y_sb", tag="y_sb")
            nc.vector.tensor_copy(y_sb[:, 0:512], ps512)
            nc.vector.tensor_copy(y_sb[:, 512:576], ps64)
            nc.sync.dma_start(out=d_y[:], in_=y_sb)

    nc.compile()
    return nc


def _get_program():
    if "nc" not in _CACHE:
        _CACHE["nc"] = _build_program()
    return _CACHE["nc"]


# ----------------------------------------------------------------------------
# host-side data prep
# ----------------------------------------------------------------------------


def _play(v, ncols):
    """length-d vector -> P-layout [128, ncols] (zero padded)."""
    v = np.asarray(v, F32).ravel()
    buf = np.zeros((ncols, 128), F32)
    buf.reshape(-1)[: v.size] = v
    return np.ascontiguousarray(buf.T)


def _prep_inputs(inputs):
    """Build the 8 per-core input maps from the full (unsharded) inputs."""
    g = {k: np.asarray(v, F32) for k, v in inputs.items()}

    common = {
        "x5": g["fw_evol_diff"].reshape(24, 1).copy(),
        "x6": g["fw_update_diff"].reshape(24, 1).copy(),
        "obs": np.concatenate(
            [g["obs_diff"], g["obs_innov_diff"]]
        ).reshape(48, 1).copy(),
        "h_q": _play(g["h_Q"], 5),
        "h_sig": _play(g["h_Sigma"], 5),
        "h_s": _play(g["h_S"], 5),
        "h_q_f": g["h_Q"].reshape(1, H).copy(),
        "h_sig_f": g["h_Sigma"].reshape(1, H).copy(),
        "h_s_f": g["h_S"].reshape(1, H).copy(),
        "w5": np.ascontiguousarray(g["W5"].T),
        "w6": np.ascontiguousarray(g["W6"].T),
        "w7": np.ascontiguousarray(g["W7"].T),
        "w1": np.ascontiguousarray(g["W1"].T),
        "b5": g["b5"].reshape(1, -1).copy(),
        "b6": g["b6"].reshape(1, -1).copy(),
        "b7": g["b7"].reshape(1, -1).copy(),
        "b1": g["b1"].reshape(1, -1).copy(),
    }
    for tag, suf in (("q", "Q"), ("sig", "Sig"), ("s", "S")):
        Wih, Whh = g[f"Wih_{suf}"], g[f"Whh_{suf}"]
        bih, bhh = g[f"bih_{suf}"], g[f"bhh_{suf}"]
        common[f"wrz_{tag}"] = np.ascontiguousarray(
            np.concatenate([Wih[0 : 2 * H], Whh[0 : 2 * H]], axis=1).T
        )
        common[f"win_{tag}"] = np.ascontiguousarray(Wih[2 * H :].T)
        common[f"whn_{tag}"] = np.ascontiguousarray(Whh[2 * H :].T)
        common[f"brz_{tag}"] = (bih[0 : 2 * H] + bhh[0 : 2 * H]).reshape(1, -1)
        common[f"bin_{tag}"] = bih[2 * H :].reshape(1, -1).copy()
        common[f"bhn_{tag}"] = bhh[2 * H :].reshape(1, -1).copy()

    in_maps = []
    for k in range(NCORES):
        m = dict(common)
        sl = slice(k * MSH, (k + 1) * MSH)
        m["w2a"] = np.ascontiguousarray(g["W2a"][sl, :].T)
        m["w2b"] = np.ascontiguousarray(g["W2b"][:, sl].T)
        m["b2a"] = g["b2a"][sl].reshape(1, -1).copy()
        in_maps.append(m)
    return in_maps


def run(trace=False, **inputs):
    from concourse.bass_utils import run_bass_kernel_spmd

    nc = _get_program()
    in_maps = _prep_inputs(inputs)
    res = run_bass_kernel_spmd(nc, in_maps, list(range(NCORES)), trace=trace)
    y = np.zeros(D2_OUT, np.float64)
    for r in res.results:
        y += r["y"].reshape(-1).astype(np.float64)
    out = (y.astype(F32) + np.asarray(inputs["b2b"], F32)).reshape(24, 24)
    return out, res


def kernel(**inputs):
    out, _ = run(trace=False, **inputs)
    return out



# revision 3
# speedup vs baseline: 1.6936x; 1.6936x over previous
"""Trainium2 Bass kernel for the KNet-style recurrent chain (batch=1).

V2 strategy (memory-bound, ~353MB fp32 weights -> ~177MB bf16):
  - All weights cast to bf16 on host; matvec stationary operands (activation
    vectors in P-layout) are bf16; PSUM/elementwise stay fp32.
  - Host pre-shuffles every weight into [128, B, M] chunk layout so each DMA
    is one contiguous >=4KB run per partition (max HBM efficiency).
  - Biases are folded into the weights as an extra K-row (the activation
    vector carries a 1.0 in the matching row), so no bias DMAs or adds.
  - The small GRU chain is replicated on all 8 cores; FC2 (W2a/W2b) is
    tensor-parallel 8-way; host sums the 8 partial y vectors + b2b.
  - Emission is split into phase A (everything that only depends on kernel
    inputs: FC5/FC6/FC7, all Whh@h gates, the rz/ gin parts fed by h/out6/
    out7) and phase B (the serial hQ->hSig->out1->hS chain), so the PE can
    run phase A back-to-back while DMA streams ahead.
  - Matvecs run weight-moving: out[1,N] += x_chunk[K,1].T @ W_chunk[K,N].
    PSUM plan: "mv" [1,1152] fp32 x2 bufs (6 banks) + "tp" [128,45] x2
    (2 banks) = all 8 banks.
"""

import sys

sys.path.insert(0, "/opt/trn_rl_repo")

import numpy as np
import ml_dtypes

NCORES = 8
H = 576
D2_HID, D2_IN, D2_OUT = 46080, 1152, 576
MSH = D2_HID // NCORES       # 5760 rows of W2a per core
NM2 = MSH // 128             # 45 h_fc chunks per core

F32 = np.float32
BF16 = ml_dtypes.bfloat16

# ---------------------------------------------------------------------------
# shared layout metadata (host pack + device emission must agree)
# ---------------------------------------------------------------------------

# segment vectors: name -> length
VDIM = {
    "x5": 24, "x6": 24, "obs": 48,
    "h_q": H, "h_sig": H, "h_s": H,
    "out5": 480, "out6": 480, "out7": 960,
    "hQ": H, "hSig": H, "out1": H, "in2": D2_IN,
}

# weight passes: name -> (segment list, m_out, has_bias_row)
WCFG = {
    "w5":        (["x5"], 480, True),
    "w6":        (["x6"], 480, True),
    "w7":        (["obs"], 960, True),
    "whn_q":     (["h_q"], H, True),
    "whn_sig":   (["h_sig"], H, True),
    "whn_s":     (["h_s"], H, True),
    "wrz_q_h":   (["h_q"], 2 * H, True),
    "wrz_sig_h": (["out6", "h_sig"], 2 * H, True),
    "wrz_s_h":   (["out7", "h_s"], 2 * H, True),
    "win_sig_h": (["out6"], H, True),
    "win_s_h":   (["out7"], H, True),
    "win_q":     (["out5"], H, True),
    "wrz_q_x":   (["out5"], 2 * H, False),
    "wrz_sig_x": (["hQ"], 2 * H, False),
    "win_sig_x": (["hQ"], H, False),
    "w1":        (["hSig"], H, True),
    "wrz_s_x":   (["out1"], 2 * H, False),
    "win_s_x":   (["out1"], H, False),
}

# FC2a output stripes (512-aligned, last is 128)
STRIPES = [(m0, min(512, MSH - m0)) for m0 in range(0, MSH, 512)]


def _chunk_meta(wname):
    """[(seg, col_in_seg, ksz, has_bias_row)] for each 128-row K chunk."""
    segs, m_out, has_bias = WCFG[wname]
    meta = []
    for seg in segs:
        d = VDIM[seg]
        nb = (d + 127) // 128
        for c in range(nb):
            meta.append([seg, c, min(128, d - c * 128), False])
    if has_bias:
        assert meta[-1][2] < 128, wname
        meta[-1][3] = True
    return meta


def _nsplits(m):
    return [(n0, min(512, m - n0)) for n0 in range(0, m, 512)]


_CACHE = {}


def _build_program():
    import concourse.bass as bass  # noqa: F401
    from concourse import bacc, mybir
    import concourse.tile as tile

    f32 = mybir.dt.float32
    bf16 = mybir.dt.bfloat16
    AF = mybir.ActivationFunctionType

    nc = bacc.Bacc(
        "TRN2", target_bir_lowering=False, debug=False, num_devices=NCORES
    )

    def din(name, shape, dt=bf16):
        return nc.dram_tensor(name, list(shape), dt, kind="ExternalInput")

    # packed activations: cols 0-4 h_q, 5-9 h_sig, 10-14 h_s (P-layout,
    # with 1.0 bias markers at row 64 of cols 4/9/14), col 15 x5 (1.0 at
    # row 24), col 16 x6, col 17 obs (1.0 at row 48)
    d_acts = din("acts", (128, 18))
    d_hf = din("hf", (1, 3 * H), f32)       # h_Q|h_Sigma|h_S free-layout

    dw = {}
    for wname in WCFG:
        meta = _chunk_meta(wname)
        dw[wname] = din(wname, (128, len(meta), WCFG[wname][1]))
    # FC2a: one dram tensor per output stripe, pre-shuffled [128, 9, nsz]
    for si, (m0, nsz) in enumerate(STRIPES):
        dw[f"w2a_{si}"] = din(f"w2a_{si}", (128, 9, nsz))
    d_b2aw = din("b2aw", (1, MSH))           # b2a as a weight row
    dw["w2b"] = din("w2b", (128, NM2, D2_OUT))
    d_y = nc.dram_tensor("y", [1, D2_OUT], f32, kind="ExternalOutput")

    with tile.TileContext(nc) as tc:
        with (
            tc.tile_pool(name="const", bufs=1) as constp,
            tc.tile_pool(name="vecs", bufs=1) as vecp,
            tc.tile_pool(name="cw", bufs=3) as swp,
            tc.tile_pool(name="fc2", bufs=3) as bigp,
            tc.tile_pool(name="ps", bufs=1, space="PSUM") as psp,
        ):
            acts = constp.tile([128, 18], bf16, name="t_acts", tag="acts")
            nc.sync.dma_start(out=acts, in_=d_acts[:])
            hf = constp.tile([1, 3 * H], f32, name="t_hf", tag="hf")
            nc.sync.dma_start(out=hf, in_=d_hf[:])
            ident = constp.tile([1, 1], f32, name="ident", tag="ident")
            nc.vector.memset(ident, 1.0)

            # P-layout vector registry: seg -> (tile, base_col)
            VEC = {
                "h_q": (acts, 0), "h_sig": (acts, 5), "h_s": (acts, 10),
                "x5": (acts, 15), "x6": (acts, 16), "obs": (acts, 17),
            }

            def mv(wname, out_name):
                """emit DMA + matmuls for one weight pass -> psum [1,1152]"""
                segs, m_out, _ = WCFG[wname]
                meta = _chunk_meta(wname)
                B = len(meta)
                d = dw[wname]
                psum = psp.tile([1, 1152], f32, name=f"ps_{out_name}",
                                tag="mv", bufs=2)
                gn = max(1, 10240 // (m_out * 2))
                pairs = []
                for g0 in range(0, B, gn):
                    g = min(gn, B - g0)
                    wt = swp.tile([128, g, m_out], bf16, tag="cw", bufs=3,
                                  name=f"w_{wname}_{g0}")
                    nc.sync.dma_start(out=wt, in_=d[:, g0:g0 + g, :])
                    for j in range(g):
                        seg, c, ksz, hasb = meta[g0 + j]
                        k = ksz + (1 if hasb else 0)
                        vt, c0 = VEC[seg]
                        pairs.append(
                            (wt[0:k, j, :], vt[0:k, c0 + c:c0 + c + 1])
                        )
                nch = len(pairs)
                for ci, (w_ap, x_ap) in enumerate(pairs):
                    for n0, nsz in _nsplits(m_out):
                        nc.tensor.matmul(
                            psum[0:1, n0:n0 + nsz],
                            x_ap,
                            w_ap[:, n0:n0 + nsz],
                            start=(ci == 0),
                            stop=(ci == nch - 1),
                            skip_group_check=True,
                        )
                return psum

            def to_play(free_ap, d, name, bias_row=None):
                """free [1,d] f32 -> P-layout bf16 [128, ncols(+1 if bias)]"""
                n_m = (d + 127) // 128
                ncols = n_m + (1 if bias_row is not None else 0)
                ps_t = psp.tile([128, 45], f32, name=f"pst_{name}",
                                tag="tp", bufs=2)
                for c in range(n_m):
                    csz = min(128, d - c * 128)
                    nc.tensor.matmul(
                        ps_t[0:csz, c:c + 1],
                        free_ap[0:1, c * 128:c * 128 + csz],
                        ident,
                        is_transpose=True,
                        start=(c == 0),
                        stop=(c == n_m - 1),
                        skip_group_check=True,
                    )
                pl = vecp.tile([128, ncols], bf16, name=name, tag=name)
                nc.vector.tensor_copy(pl[:, 0:n_m], ps_t[:, 0:n_m])
                if bias_row is not None:
                    # rows past the marker are never read (slices stop at
                    # ksz/ksz+1), so only the single element needs setting
                    r, c = bias_row
                    nc.vector.memset(pl[r:r + 1, c:c + 1], 1.0)
                return pl

            def act_out(psum, m, name, func, tag=None, bufs=1):
                """out = func(psum[0:1,0:m]) -> sbuf f32 (one Scalar inst)"""
                out = vecp.tile([1, m], f32, name=name, tag=tag or name,
                                bufs=bufs)
                nc.scalar.activation(out, psum[0:1, 0:m], func)
                return out

            def copy_out(psum, m, name, tag=None):
                out = vecp.tile([1, m], f32, name=name, tag=tag or name)
                nc.vector.tensor_copy(out, psum[0:1, 0:m])
                return out

            # ---------------- phase A ----------------
            ps = mv("w5", "out5")
            out5_f = act_out(ps, 480, "out5_f", AF.Relu, tag="vf", bufs=2)
            VEC["out5"] = (to_play(out5_f, 480, "out5P", bias_row=(96, 3)), 0)

            ps = mv("w6", "out6")
            out6_f = act_out(ps, 480, "out6_f", AF.Relu, tag="vf", bufs=2)
            VEC["out6"] = (to_play(out6_f, 480, "out6P", bias_row=(96, 3)), 0)

            ps = mv("w7", "out7")
            out7_f = act_out(ps, 960, "out7_f", AF.Relu, tag="vf", bufs=2)
            VEC["out7"] = (to_play(out7_f, 960, "out7P", bias_row=(64, 7)), 0)

            ghn = {}
            for g in ("q", "sig", "s"):
                ghn[g] = copy_out(mv(f"whn_{g}", f"ghn_{g}"), H, f"ghn_{g}")
            rzh = {}
            for g in ("q", "sig", "s"):
                rzh[g] = copy_out(mv(f"wrz_{g}_h", f"rzh_{g}"), 2 * H,
                                  f"rzh_{g}")
            ginh = {}
            for g in ("sig", "s"):
                ginh[g] = copy_out(mv(f"win_{g}_h", f"ginh_{g}"), H,
                                   f"ginh_{g}")
            gin_q = copy_out(mv("win_q", "gin_q"), H, "gin_q", tag="gin")

            in2_f = vecp.tile([1, D2_IN], f32, name="in2_f", tag="in2_f")

            def gru_elem(g, ps_rz, gin, hf_off, out_ap, out_name):
                """rz psum + precomputed parts -> h' (f32) into out_ap"""
                rz = vecp.tile([1, 2 * H], f32, name=f"rz_{g}", tag="rz",
                               bufs=2)
                nc.vector.tensor_add(rz, ps_rz[0:1, 0:2 * H], rzh[g])
                nc.scalar.activation(rz, rz, AF.Sigmoid)
                t3 = vecp.tile([1, H], f32, name=f"t3_{g}", tag="t3")
                nc.vector.tensor_mul(t3, rz[0:1, 0:H], ghn[g])
                nc.vector.tensor_add(t3, gin, t3)
                n_t = vecp.tile([1, H], f32, name=f"n_{g}", tag="n_t")
                nc.scalar.activation(n_t, t3, AF.Tanh)
                t5 = vecp.tile([1, H], f32, name=f"t5_{g}", tag="t5")
                nc.vector.tensor_sub(t5, hf[0:1, hf_off:hf_off + H], n_t)
                nc.vector.tensor_mul(t5, rz[0:1, H:2 * H], t5)
                nc.vector.tensor_add(out_ap, n_t, t5)

            # GRU_Q (x = out5, available in phase A)
            ps_rz = mv("wrz_q_x", "rzx_q")
            hQ_f = vecp.tile([1, H], f32, name="hQ_f", tag="hQ_f")
            gru_elem("q", ps_rz, gin_q, 0, hQ_f, "hQ")
            VEC["hQ"] = (to_play(hQ_f, H, "hQP", bias_row=(64, 4)), 0)

            # ---------------- phase B ----------------
            # GRU_Sigma (x = [hQ, out6])
            ps_rz = mv("wrz_sig_x", "rzx_sig")
            ps_gin = mv("win_sig_x", "ginx_sig")
            gin = vecp.tile([1, H], f32, name="gin_sig", tag="gin")
            nc.vector.tensor_add(gin, ps_gin[0:1, 0:H], ginh["sig"])
            gru_elem("sig", ps_rz, gin, H, in2_f[0:1, 0:H], "hSig")
            VEC["hSig"] = (
                to_play(in2_f[0:1, 0:H], H, "hSigP", bias_row=(64, 4)), 0
            )

            # FC1
            ps = mv("w1", "out1")
            out1_f = act_out(ps, H, "out1_f", AF.Relu, tag="vf", bufs=2)
            VEC["out1"] = (to_play(out1_f, H, "out1P", bias_row=(64, 4)), 0)

            # GRU_S (x = [out1, out7])
            ps_rz = mv("wrz_s_x", "rzx_s")
            ps_gin = mv("win_s_x", "ginx_s")
            gin = vecp.tile([1, H], f32, name="gin_s", tag="gin")
            nc.vector.tensor_add(gin, ps_gin[0:1, 0:H], ginh["s"])
            gru_elem("s", ps_rz, gin, 2 * H, in2_f[0:1, H:2 * H], "hS")

            # in2 -> P-layout [128, 10] (9 data cols + bias col)
            in2P = to_play(in2_f, D2_IN, "in2P", bias_row=(0, 9))

            # ---------------- FC2a ----------------
            b2aw = constp.tile([1, MSH], bf16, name="t_b2aw", tag="b2aw")
            nc.sync.dma_start(out=b2aw, in_=d_b2aw[:])
            h_fc = vecp.tile([128, NM2], bf16, name="h_fc", tag="h_fc")
            for si, (m0, nsz) in enumerate(STRIPES):
                wt = bigp.tile([128, 9, nsz], bf16, tag="fca",
                               name=f"w2a_{si}", bufs=3)
                nc.sync.dma_start(out=wt, in_=dw[f"w2a_{si}"][:])
                psf = psp.tile([1, 1152], f32, name=f"ps_f{si}", tag="mv",
                               bufs=2)
                for ci in range(9):
                    nc.tensor.matmul(
                        psf[0:1, 0:nsz],
                        in2P[0:128, ci:ci + 1],
                        wt[:, ci, :],
                        start=(ci == 0),
                        stop=False,
                        skip_group_check=True,
                    )
                nc.tensor.matmul(
                    psf[0:1, 0:nsz],
                    in2P[0:1, 9:10],
                    b2aw[0:1, m0:m0 + nsz],
                    start=False,
                    stop=True,
                    skip_group_check=True,
                )
                hstr = vecp.tile([1, 512], f32, name=f"hstr_{si}",
                                 tag="hstr", bufs=2)
                nc.scalar.activation(
                    hstr[0:1, 0:nsz], psf[0:1, 0:nsz], AF.Relu
                )
                ps_t = psp.tile([128, 45], f32, name=f"pst_fc{si}",
                                tag="tp", bufs=2)
                ncol = nsz // 128
                for c in range(ncol):
                    nc.tensor.matmul(
                        ps_t[:, c:c + 1],
                        hstr[0:1, c * 128:(c + 1) * 128],
                        ident,
                        is_transpose=True,
                        start=(c == 0),
                        stop=(c == ncol - 1),
                        skip_group_check=True,
                    )
                col0 = m0 // 128
                nc.vector.tensor_copy(
                    h_fc[:, col0:col0 + ncol], ps_t[:, 0:ncol]
                )

            # ---------------- FC2b ----------------
            ps_y = psp.tile([1, 1152], f32, name="ps_y", tag="mv", bufs=2)
            W2B_GRP = 9
            for g0 in range(0, NM2, W2B_GRP):
                g = min(W2B_GRP, NM2 - g0)
                wt = bigp.tile([128, g, D2_OUT], bf16, tag="w2b",
                               name=f"w2b_{g0}", bufs=2)
                nc.sync.dma_start(out=wt, in_=dw["w2b"][:, g0:g0 + g, :])
                for j in range(g):
                    kb = g0 + j
                    lhs = h_fc[:, kb:kb + 1]
                    nc.tensor.matmul(
                        ps_y[0:1, 0:512], lhs, wt[:, j, 0:512],
                        start=(kb == 0), stop=(kb == NM2 - 1),
                        skip_group_check=True,
                    )
                    nc.tensor.matmul(
                        ps_y[0:1, 512:576], lhs, wt[:, j, 512:576],
                        start=(kb == 0), stop=(kb == NM2 - 1),
                        skip_group_check=True,
                    )
            y_sb = constp.tile([1, D2_OUT], f32, name="y_sb", tag="y_sb")
            nc.vector.tensor_copy(y_sb, ps_y[0:1, 0:D2_OUT])
            nc.sync.dma_start(out=d_y[:], in_=y_sb)

    nc.compile()
    return nc


def _get_program():
    if "nc" not in _CACHE:
        _CACHE["nc"] = _build_program()
    return _CACHE["nc"]


# ---------------------------------------------------------------------------
# host-side data prep
# ---------------------------------------------------------------------------


def _pack_w(wname, WT, bias):
    """WT [K, M] f32 (+bias [M]) -> chunked [128, B, M] bf16 per layout."""
    meta = _chunk_meta(wname)
    M = WT.shape[1]
    buf = np.zeros((128, len(meta), M), dtype=BF16)
    row = 0
    for i, (seg, c, ksz, hasb) in enumerate(meta):
        buf[0:ksz, i, :] = WT[row:row + ksz].astype(BF16)
        row += ksz
        if hasb:
            buf[ksz, i, :] = bias.astype(BF16)
    assert row == WT.shape[0]
    return buf


def _prep_inputs(inputs):
    g = {k: np.asarray(v, F32) for k, v in inputs.items()}

    # packed activations
    acts = np.zeros((128, 18), dtype=BF16)
    for ci, h in ((0, g["h_Q"]), (5, g["h_Sigma"]), (10, g["h_S"])):
        buf = np.zeros((5, 128), F32)
        buf.reshape(-1)[:H] = h
        acts[:, ci:ci + 5] = buf.T.astype(BF16)
        acts[64, ci + 4] = BF16(1.0)
    acts[0:24, 15] = g["fw_evol_diff"].astype(BF16)
    acts[24, 15] = BF16(1.0)
    acts[0:24, 16] = g["fw_update_diff"].astype(BF16)
    acts[24, 16] = BF16(1.0)
    obs = np.concatenate([g["obs_diff"], g["obs_innov_diff"]])
    acts[0:48, 17] = obs.astype(BF16)
    acts[48, 17] = BF16(1.0)

    common = {
        "acts": acts,
        "hf": np.concatenate(
            [g["h_Q"], g["h_Sigma"], g["h_S"]]
        ).reshape(1, -1).astype(F32),
    }

    common["w5"] = _pack_w("w5", g["W5"].T.copy(), g["b5"])
    common["w6"] = _pack_w("w6", g["W6"].T.copy(), g["b6"])
    common["w7"] = _pack_w("w7", g["W7"].T.copy(), g["b7"])
    common["w1"] = _pack_w("w1", g["W1"].T.copy(), g["b1"])

    for tag, suf in (("q", "Q"), ("sig", "Sig"), ("s", "S")):
        Wih, Whh = g[f"Wih_{suf}"], g[f"Whh_{suf}"]
        bih, bhh = g[f"bih_{suf}"], g[f"bhh_{suf}"]
        brz = bih[0:2 * H] + bhh[0:2 * H]
        common[f"whn_{tag}"] = _pack_w(
            f"whn_{tag}", Whh[2 * H:].T.copy(), bhh[2 * H:])
        if tag == "q":
            # x = out5 (480)
            common["wrz_q_h"] = _pack_w("wrz_q_h", Whh[0:2 * H].T.copy(), brz)
            common["wrz_q_x"] = _pack_w("wrz_q_x", Wih[0:2 * H].T.copy(), None)
            common["win_q"] = _pack_w(
                "win_q", Wih[2 * H:].T.copy(), bih[2 * H:])
        else:
            # x = [chain_vec (576), early_vec (480 or 960)]
            xd = H
            common[f"wrz_{tag}_h"] = _pack_w(
                f"wrz_{tag}_h",
                np.concatenate(
                    [Wih[0:2 * H, xd:].T, Whh[0:2 * H].T], axis=0
                ).copy(),
                brz,
            )
            common[f"wrz_{tag}_x"] = _pack_w(
                f"wrz_{tag}_x", Wih[0:2 * H, 0:xd].T.copy(), None)
            common[f"win_{tag}_h"] = _pack_w(
                f"win_{tag}_h", Wih[2 * H:, xd:].T.copy(), bih[2 * H:])
            common[f"win_{tag}_x"] = _pack_w(
                f"win_{tag}_x", Wih[2 * H:, 0:xd].T.copy(), None)

    in_maps = []
    for k in range(NCORES):
        m = dict(common)
        sl = slice(k * MSH, (k + 1) * MSH)
        W2aT = np.ascontiguousarray(g["W2a"][sl, :].T)   # [1152, MSH]
        for si, (m0, nsz) in enumerate(STRIPES):
            blk = np.zeros((128, 9, nsz), dtype=BF16)
            for b in range(9):
                blk[:, b, :] = W2aT[b * 128:(b + 1) * 128,
                                    m0:m0 + nsz].astype(BF16)
            m[f"w2a_{si}"] = blk
        m["b2aw"] = g["b2a"][sl].reshape(1, -1).astype(BF16)
        W2bT = np.ascontiguousarray(g["W2b"][:, sl].T)   # [MSH, 576]
        blk = np.zeros((128, NM2, D2_OUT), dtype=BF16)
        for b in range(NM2):
            blk[:, b, :] = W2bT[b * 128:(b + 1) * 128, :].astype(BF16)
        m["w2b"] = blk
        in_maps.append(m)
    return in_maps


def run(trace=False, **inputs):
    from concourse.bass_utils import run_bass_kernel_spmd

    nc = _get_program()
    in_maps = _prep_inputs(inputs)
    res = run_bass_kernel_spmd(nc, in_maps, list(range(NCORES)), trace=trace)
    y = np.zeros(D2_OUT, np.float64)
    for r in res.results:
        y += r["y"].reshape(-1).astype(np.float64)
    out = (y.astype(F32) + np.asarray(inputs["b2b"], F32)).reshape(24, 24)
    return out, res


def kernel(**inputs):
    out, _ = run(trace=False, **inputs)
    return out
